# revision 1
# baseline (speedup 1.0000x reference)
"""Bicubic 4x downsample (MATLAB imresize-style) on Trainium2, 8-core data parallel.

Math: the reference is a separable resize: H-resize then W-resize, each a
gather + weighted sum along one axis. Both are linear maps, so per channel
image X [H, W]:

    out = WH @ X @ WW^T,   WH [OH, H] (banded),  WW [OW, W] (banded)

We build the dense banded matrices host-side from (w_h, idx_h, w_w, idx_w)
(boundary reflection folds in for free) and evaluate both contractions on
the PE array, using the image as the *stationary* (weights) operand so each
image element streams through the PE exactly once:

  stage 1:  out1T[w, oh] = sum_h  X[h, w]    * WHT[h, oh]   (lhsT = X tile)
  stage 2:  out2 [oh,ow] = sum_w  out1T[w,oh]* WWT[w, ow]   (lhsT = out1T)

Both stages exploit the band structure: an h-tile of 128 input rows only
contributes to a ~35-wide window of output rows, so the moving operand is a
narrow slice of the packed weight matrix. PSUM per-element has_written bits
accumulate overlapping windows across tiles.

Sharding: pure data parallel, batch b -> core b (8 batches, 8 cores).
"""

import numpy as np

TILE = 128


def _ensure_concourse():
    try:
        import concourse  # noqa: F401
    except ImportError:
        import sys
        for p in ("/opt/trn_rl_repo", "/root/.axon_site/_ro/trn_rl_repo"):
            if p not in sys.path:
                sys.path.insert(0, p)


_PATCHED = False


def _patch_tile_drain():
    """This walrus build rejects >1 sem wait on TPB_CTRL instructions (the
    Tile exit Drain). Split the final drain's waits into single-wait nops."""
    global _PATCHED
    if _PATCHED:
        return
    from concourse import tile
    from concourse.vector_clock import VectorClock, ScopedClock

    def _drain_and_barrier(self, tick_clock, wait_clock):
        gc = tick_clock.global_clock
        n = len(gc)
        for i in range(n):
            if gc[i] <= 0:
                continue
            vc = VectorClock([gc[j] if j == i else 0 for j in range(n)])
            nop_inst = self.nc.sync.nop(nofuse=True, hint="drain_split")
            wait_clock.add_sem_waits(nop_inst.ins, ScopedClock({None: vc}))
        self.nc.sync.drain()
        self.nc.all_engine_barrier()
        assert self.sems is not None
        popped = self.nc._tile_sem_poison_stack.pop()
        assert popped is self._sem_poison
        self.nc.clear_and_free_semaphores(list(self.sems.allocated().values()))
        self.nc.all_engine_barrier()

    tile.TileContext._drain_and_barrier = _drain_and_barrier
    _PATCHED = True


def _split_multi_waits(nc):
    """This walrus build rejects instructions carrying >1 sem wait. Hoist all
    but the last wait of any instruction onto same-engine nops placed
    immediately before it (engine streams execute block order in-order, so
    waiting on a preceding nop is equivalent)."""
    from concourse import mybir

    uid = 0
    for fn in nc.m.functions:
        for bb in fn.blocks:
            insts = bb.instructions  # live list
            new_list = []
            changed = False
            for ins in list(insts):
                si = ins.sync_info
                if si is not None and len(si.on_wait) > 1:
                    waits = list(si.on_wait)
                    for wt in waits[:-1]:
                        uid += 1
                        nop = mybir.InstNoOp(
                            name=f"ws_nop_{uid}",
                            engine=ins.engine,
                            ins=[],
                            outs=[],
                            sync_info=mybir.SyncInfo(on_wait=[wt], on_update=[]),
                            bass_nofuse=True,
                        )
                        new_list.append(nop)
                    ins.sync_info = mybir.SyncInfo(
                        on_wait=[waits[-1]], on_update=list(si.on_update)
                    )
                    changed = True
                new_list.append(ins)
            if changed:
                insts.clear()
                insts.extend(new_list)


def _dense_t(weights, indices, in_len):
    """Dense transposed resize matrix [in_len, out_len]:
    M[i, o] = sum over taps p with indices[o, p] == i of weights[o, p]."""
    w = np.asarray(weights, np.float32)
    idx = np.asarray(indices, np.int64)
    out_len, ntap = w.shape
    m = np.zeros((in_len, out_len), np.float32)
    ocol = np.repeat(np.arange(out_len), ntap)
    np.add.at(m, (idx.ravel(), ocol), w.ravel())
    return m


def _windows(mat_t):
    """Per 128-row tile of the [in, out] matrix: (out_lo, out_hi, packed_off).
    Band structure makes the nonzero columns of each tile contiguous-ish;
    we take the [first, last+1] span (interior zeros just add zeros)."""
    wins = []
    off = 0
    for t0 in range(0, mat_t.shape[0], TILE):
        blk = mat_t[t0:t0 + TILE]
        nz = np.flatnonzero(np.any(blk != 0.0, axis=0))
        lo, hi = int(nz[0]), int(nz[-1]) + 1
        wins.append((lo, hi, off))
        off += hi - lo
    return wins, off


def _pack(mat_t, wins):
    total = wins[-1][2] + (wins[-1][1] - wins[-1][0])
    p = np.zeros((TILE, total), np.float32)
    for (lo, hi, off), t0 in zip(wins, range(0, mat_t.shape[0], TILE)):
        blk = mat_t[t0:t0 + TILE, lo:hi]
        p[:blk.shape[0], off:off + (hi - lo)] = blk
    return p


def _oh_chunks(n):
    return [(a, min(a + TILE, n)) for a in range(0, n, TILE)]


def _build_program(C, H, W, OH, OW, hwins, wwins, W1, W2, repeat=1, mode="full", ps1_bufs=2, ps2_mult=2):
    from concourse import bass, tile, mybir

    f32 = mybir.dt.float32
    nc = bass.Bass()
    x_d = nc.declare_dram_parameter("x", [C, H, W], f32, isOutput=False)
    wht_d = nc.declare_dram_parameter("wht", [TILE, W1], f32, isOutput=False)
    wwt_d = nc.declare_dram_parameter("wwt", [TILE, W2], f32, isOutput=False)
    out_d = nc.declare_dram_parameter("out", [C, OH, OW], f32, isOutput=True)

    HT = (H + TILE - 1) // TILE
    WT = (W + TILE - 1) // TILE
    ohc = _oh_chunks(OH)

    with tile.TileContext(nc) as tc:
        with (
            tc.tile_pool(name="consts", bufs=1) as cpool,
            tc.tile_pool(name="xch", bufs=2) as xpool,
            tc.tile_pool(name="o1", bufs=17) as o1pool,
            tc.tile_pool(name="osb", bufs=2) as opool,
            tc.tile_pool(name="ps1", bufs=ps1_bufs, space=bass.MemorySpace.PSUM) as ps1pool,
            tc.tile_pool(name="ps2", bufs=ps2_mult * len(ohc), space=bass.MemorySpace.PSUM) as ps2pool,
        ):
            wht_sb = cpool.tile([TILE, W1], f32)
            nc.sync.dma_start(out=wht_sb[:, :], in_=wht_d[:, :])
            wwt_sb = cpool.tile([TILE, W2], f32)
            nc.scalar.dma_start(out=wwt_sb[:, :], in_=wwt_d[:, :])

            o1_dummy = None
            if mode == "nocopy":
                o1_dummy = cpool.tile([TILE, OH], f32, name="o1_dummy")
                nc.gpsimd.memset(o1_dummy[:, :], 0.0)
            for rc in range(repeat * C):
                c = rc % C
                # whole channel resident: [128, HT*W], h-tile ht at free
                # offset ht*W (row-major rows are contiguous in DRAM)
                xc = xpool.tile([TILE, HT * W], f32)
                for ht in range(HT):
                    p = min(TILE, H - TILE * ht)
                    eng = nc.sync if (mode == "dsp" or ht % 2 == 0) else nc.scalar
                    eng.dma_start(
                        out=xc[0:p, ht * W:ht * W + W],
                        in_=x_d[c, TILE * ht:TILE * ht + p, :],
                    )

                if mode == "dma":
                    for k, (a, b) in enumerate(ohc):
                        osb = opool.tile([TILE, OW], f32)
                        nc.vector.tensor_copy(osb[0:b - a, :], xc[0:b - a, 0:OW])
                        eng = nc.sync if k % 2 == 0 else nc.scalar
                        eng.dma_start(out=out_d[c, a:b, :], in_=osb[0:b - a, :])
                    continue
                ps2s = [ps2pool.tile([TILE, OW], f32, name="ps2", tag="ps2") for _ in ohc]

                def s2_one(wt, o1, pw):
                    wlo, whi, woff = wwins[wt]
                    for k, (a, b) in enumerate(ohc):
                        nc.tensor.matmul(
                            ps2s[k][0:b - a, wlo:whi],
                            o1[0:pw, a:b],
                            wwt_sb[0:pw, woff:woff + (whi - wlo)],
                            start=(wt == 0),
                            stop=(wt == WT - 1),
                        )

                # stage 1 for the whole channel; buffer all o1 tiles
                o1s = []
                for wt in range(WT):
                    pw = min(TILE, W - TILE * wt)
                    ps1 = ps1pool.tile([TILE, OH], f32)
                    # stage 1: out1T[w, oh] += X[h, w] * WHT[h, oh]
                    for ht in range(HT):
                        p = min(TILE, H - TILE * ht)
                        lo, hi, off = hwins[ht]
                        nc.tensor.matmul(
                            ps1[0:pw, lo:hi],
                            xc[0:p, ht * W + TILE * wt:ht * W + TILE * wt + pw],
                            wht_sb[0:p, off:off + (hi - lo)],
                            start=(ht == 0),
                            stop=(ht == HT - 1),
                        )
                    if mode == "s1":
                        continue
                    if mode == "nocopy":
                        o1 = o1_dummy
                    else:
                        o1 = o1pool.tile([TILE, OH], f32)
                        if mode == "cpalt" and wt % 2 == 1:
                            nc.scalar.copy(o1[0:pw, :], ps1[0:pw, :])
                        else:
                            nc.vector.tensor_copy(o1[0:pw, :], ps1[0:pw, :])
                    o1s.append((o1, pw))
                    if mode == "s2il" and len(o1s) >= 2:
                        s2_one(wt - 1, *o1s[wt - 1])
                if mode == "s2il":
                    s2_one(WT - 1, *o1s[WT - 1])
                # stage 2: out2[oh, ow] += out1T[w, oh] * WWT[w, ow]
                # bank-major order: all 15 w-tiles of one PSUM bank back-to-back
                # so PE never drains between accumulation-group switches
                if mode not in ("s1", "nos2", "s2il"):
                    for k, (a, b) in enumerate(ohc):
                        for wt in range(WT):
                            o1, pw = o1s[wt]
                            wlo, whi, woff = wwins[wt]
                            nc.tensor.matmul(
                                ps2s[k][0:b - a, wlo:whi],
                                o1[0:pw, a:b],
                                wwt_sb[0:pw, woff:woff + (whi - wlo)],
                                start=(wt == 0),
                                stop=(wt == WT - 1),
                            )
                for k, (a, b) in enumerate(ohc):
                    osb = opool.tile([TILE, OW], f32)
                    if mode in ("s1", "nos2"):
                        nc.vector.tensor_copy(osb[0:b - a, :], xc[0:b - a, 0:OW])
                    elif mode == "odve":
                        nc.vector.tensor_copy(osb[0:b - a, :], ps2s[k][0:b - a, :])
                    else:
                        # ACT for output copies: frees DVE for the o1 chain and
                        # unblocks ps2 bank reuse sooner (~100us/body measured)
                        nc.scalar.copy(osb[0:b - a, :], ps2s[k][0:b - a, :])
                    eng = nc.sync if (mode == "osp" or k % 2 == 0) else nc.scalar
                    eng.dma_start(out=out_d[c, a:b, :], in_=osb[0:b - a, :])

    _split_multi_waits(nc)
    return nc


def kernel(x, w_h, idx_h, w_w, idx_w, _trace=False):
    _ensure_concourse()
    _patch_tile_drain()
    from concourse.bass_utils import run_bass_kernel_spmd

    x = np.ascontiguousarray(np.asarray(x, np.float32))
    B, C, H, W = x.shape
    wht_t = _dense_t(w_h, idx_h, H)   # [H, OH]
    wwt_t = _dense_t(w_w, idx_w, W)   # [W, OW]
    OH, OW = wht_t.shape[1], wwt_t.shape[1]

    hwins, W1 = _windows(wht_t)
    wwins, W2 = _windows(wwt_t)
    wht_packed = _pack(wht_t, hwins)
    wwt_packed = _pack(wwt_t, wwins)

    nc = _build_program(C, H, W, OH, OW, hwins, wwins, W1, W2)

    in_maps = [
        {"x": x[b], "wht": wht_packed, "wwt": wwt_packed} for b in range(B)
    ]
    res = run_bass_kernel_spmd(nc, in_maps, list(range(B)), trace=bool(_trace))
    out = np.stack([res.results[i]["out"] for i in range(B)], axis=0)
    if _trace:
        return out, res
    return out

